# revision 38
# baseline (speedup 1.0000x reference)
"""MultiHeadAttention TRN2 kernel — wire-optimized, tensor-parallel over heads.

Math (B=2, H=16, S=2048, D=128, F=256, DIM=2048):
  Q = einsum('bhsf,hfd', q, Wq) + bq ; K likewise ; V = einsum('bhse,hed', v, Wv) + bv
  P = softmax(Q K^T / 16) ; o = P V ; out = concat_h(o) @ Wo + bo

The axon tunnel (~30-90 MB/s) dominates wall time, so bytes on the wire
are minimized (~42 MB up incl. donated output zeros, ~8.4 MB down, vs
~480 MB/call for a naive fp32 kernel). The cheap QKV projections
(10.7 GFLOP) run on the host via BLAS into persistent buffers, with
bias-add + layout + fp8/bf16 casts fused in one jax-cpu jit (XLA's
vectorized fp8 convert is ~3x faster than ml_dtypes.astype); the
compute-heavy attention core and the Wo projection (103 GFLOP) run on
the 8 NeuronCores.

Wire format choices (all validated against a numpy emulation first,
total rel err 8.8e-3 vs the 2e-2 gate):
- Q^T,K^T,V upload in fp8-e3m4 (range +-15.5 is ample for |Q|,|K|,|V|
  <= 3.2; 1 more mantissa bit than e4m3). Softmax normalization cancels
  the common-mode score error, so fp8 Q,K costs only ~1e-3.
- Each core uploads HALF its heads' Wo rows in bf16; a 2-core AllGather
  over pairs (c, c+4) (same heads, other batch) reconstructs the rest on
  device, so Wo crosses the wire exactly once. fp8 Wo was tested and
  rejected (4.3e-2 rel err — Wo errors hit the output linearly).
- Output is int8 with a per-row scale (absmax/126): the DVE f32->int8
  copy rounds to nearest (verified on hw vs emulation: 8.3e-3, not
  truncation's 1.6e-2), halving both the download and the donated-zeros
  upload vs bf16.

Sharding: core c -> batch b=c//4, heads (c%4)*4 .. +4 (tensor parallel
over H). Per head, per 512-query chunk: scores^T tile [128k,512q] =
KT_chunk^T @ QT_chunk on PE (fp8), exp on ACT (scale=1/16; no
max-subtraction needed, |scores|/16 <~ 1.3), P@V and row-sums accumulated
on PE over 16 k-chunks, reciprocal+normalize on DVE. Software-pipelined:
the score matmul for chunk kt+1 is queued before the exp of chunk kt is
consumed, keeping ACT (the attention bottleneck) fed.

Wo stage on device: P_c = concat_j(o_j) @ Wo[core's head rows] (PSUM
accumulation over the 4 heads), then a 4-core f32 ReduceScatter per batch
group ([[0,1,2,3],[4,5,6,7]]): core c ends with rows 512*(c%4)..+512 of
its batch's projection (sans bo), quantized int8 for download. The host
only stitches the 8 quarters, dequantizes, and adds bo.

The jax persistent compilation cache is enabled so warm calls skip the
~0.35 s/call neuronx hook (BIR verify + DVE-table regen + walrus): each
run_bass_kernel_spmd call jits a fresh closure, but the cache is keyed on
the stable HLO hash.

Device layout per core (head j = 0..3):
  qkv  [3,4,128,2048] fp8e3m4 : stacked Q^T / K^T / V-chunked per head;
                                vc[j][p, kt*128+d] = V[kt*128+p, d]
  woh  [2,128,2048]   bf16    : half of the core's heads' Wo rows (d, n)
  outq [512,2052]     int8    : quarter of the batch's out rows, plus the
                                f32 per-row absmax bitcast into the 4
                                trailing bytes (merged outputs/operands:
                                each separate jit output costs a flat
                                ~75 ms fetch round-trip over axon)
"""

import sys

import numpy as np

B, H, S, D, F = 2, 16, 2048, 128, 256
DIM = H * D
NC = 8
HPC = 4
SC512 = S // 512  # 4
NKT = S // 128  # 16
RS = S // 4  # 512 rows per core after reduce-scatter

_BUILT = None
TRACE = False
LAST_RESULTS = None


def _import_concourse():
    try:
        import concourse.bass  # noqa: F401
    except ImportError:
        sys.path.insert(0, "/opt/trn_rl_repo")


def _build():
    _import_concourse()
    from contextlib import ExitStack

    import concourse.bass as bass
    import concourse.mybir as mybir
    import concourse.tile as tile

    f32 = mybir.dt.float32
    bf16 = mybir.dt.bfloat16
    fp8 = mybir.dt.float8e3
    AF = mybir.ActivationFunctionType

    nc = bass.Bass(target_bir_lowering=False, num_devices=NC)

    # qT/kT/vc stacked in ONE tensor (fewer jit operands: each extra
    # operand/output costs fixed dispatch+fetch latency over axon)
    qkv_d = nc.dram_tensor("qkv", [HPC, 3, 128, S], fp8, kind="ExternalInput")
    # each core uploads HALF its heads' Wo rows (cores 0-3: j={0,1};
    # cores 4-7: j={2,3}); pairs (c, c+4) share a head group, so a 2-core
    # AllGather reconstructs all 4 heads' rows on device, halving the upload
    woh_d = nc.dram_tensor("woh", [HPC // 2, 128, DIM], bf16, kind="ExternalInput")
    # int8 payload + the f32 per-row absmax bitcast into 4 trailing int8
    # columns (a separate tiny output costs a flat ~75 ms fetch round-trip)
    out_d = nc.dram_tensor("outq", [RS, DIM + 4], mybir.dt.int8,
                           kind="ExternalOutput")

    with ExitStack() as ctx:
        tc = ctx.enter_context(tile.TileContext(nc))
        consts = ctx.enter_context(tc.tile_pool(name="consts", bufs=1))
        heads = ctx.enter_context(tc.tile_pool(name="heads", bufs=2))
        sm = ctx.enter_context(tc.tile_pool(name="sm", bufs=2))
        otn_pool = ctx.enter_context(tc.tile_pool(name="otn", bufs=4))
        wop = ctx.enter_context(tc.tile_pool(name="wop", bufs=4))
        pout = ctx.enter_context(tc.tile_pool(name="pout", bufs=3))
        dram = ctx.enter_context(tc.tile_pool(name="dram", bufs=1, space="DRAM"))
        ps = ctx.enter_context(tc.tile_pool(name="ps", bufs=1, space="PSUM"))

        ones_sb = consts.tile([128, 128], bf16)
        nc.vector.memset(ones_sb[:], 1.0)

        woh_b = dram.tile([HPC // 2, 128, DIM], bf16)
        nc.gpsimd.dma_start(out=woh_b[:], in_=woh_d[:])
        wo_gath = dram.tile([HPC, 128, DIM], bf16)
        nc.gpsimd.collective_compute(
            "AllGather",
            mybir.AluOpType.bypass,
            replica_groups=[[0, 4], [1, 5], [2, 6], [3, 7]],
            ins=[woh_b.opt()],
            outs=[wo_gath.opt()],
        )
        wo_sb = []
        for j in range(HPC):
            w = wop.tile([128, DIM], bf16, tag="wo", name=f"wo{j}")
            nc.scalar.dma_start(out=w, in_=wo_gath[j])
            wo_sb.append(w)

        P_t = dram.tile([S, DIM], f32)
        R_t = dram.tile([RS, DIM], f32)

        def emit_loads(j):
            qt = heads.tile([128, S], fp8, tag="qt", name=f"qt{j}")
            nc.sync.dma_start(out=qt, in_=qkv_d[j, 0])
            kt = heads.tile([128, S], fp8, tag="kt", name=f"kt{j}")
            nc.gpsimd.dma_start(out=kt, in_=qkv_d[j, 1])
            vc = heads.tile([128, S], fp8, tag="vc", name=f"vc{j}")
            nc.scalar.dma_start(out=vc, in_=qkv_d[j, 2])
            return qt, kt, vc

        store_q = [nc.gpsimd, nc.sync, nc.scalar]
        nst = 0
        otn = []

        cur_loads = emit_loads(0)
        for j in range(HPC):
            QT, KT, Vc = cur_loads
            if j + 1 < HPC:
                cur_loads = emit_loads(j + 1)
            oT = otn_pool.tile([128, S], bf16, tag="otn", name=f"oTn{j}")
            otn.append(oT)
            for qc in range(SC512):
                qsl = slice(qc * 512, (qc + 1) * 512)
                po = ps.tile([128, 512], f32, tag="o", bufs=2, name=f"po{j}_{qc}")
                pr = ps.tile([128, 512], f32, tag="r", bufs=2, name=f"pr{j}_{qc}")

                def emit_pscore(kt_i):
                    csl = slice(kt_i * 128, (kt_i + 1) * 128)
                    t = ps.tile([128, 512], f32, tag="s", bufs=3,
                                name=f"ps{j}_{qc}_{kt_i}")
                    nc.tensor.matmul(t, KT[:, csl], QT[:, qsl],
                                     start=True, stop=True)
                    return t

                cur = emit_pscore(0)
                for kt_i in range(NKT):
                    csl = slice(kt_i * 128, (kt_i + 1) * 128)
                    pT = sm.tile([128, 512], bf16, tag="pT", bufs=3,
                                 name=f"pT{j}_{qc}_{kt_i}")
                    nc.scalar.activation(out=pT, in_=cur, func=AF.Exp,
                                         bias=0.0, scale=0.0625)
                    if kt_i + 1 < NKT:
                        cur = emit_pscore(kt_i + 1)
                    nc.tensor.matmul(po, Vc[:, csl], pT,
                                     start=(kt_i == 0), stop=(kt_i == NKT - 1))
                    nc.tensor.matmul(pr, ones_sb, pT,
                                     start=(kt_i == 0), stop=(kt_i == NKT - 1))
                rr = sm.tile([128, 512], f32, tag="rr", bufs=2, name=f"rr{j}_{qc}")
                nc.vector.reciprocal(out=rr, in_=pr)
                nc.vector.tensor_mul(out=oT[:, qsl], in0=po, in1=rr)

        # Wo partial: P[sc*128:+128, dc*512:+512] = sum_j oT_j[:,ssl]^T @ wo_j[:,dsl]
        for sc in range(S // 128):
            ssl = slice(sc * 128, (sc + 1) * 128)
            for dc in range(DIM // 512):
                dsl = slice(dc * 512, (dc + 1) * 512)
                pp = ps.tile([128, 512], f32, tag="s", bufs=3, name=f"pp{sc}_{dc}")
                for j in range(HPC):
                    nc.tensor.matmul(pp, otn[j][:, ssl], wo_sb[j][:, dsl],
                                     start=(j == 0), stop=(j == HPC - 1))
                ow = pout.tile([128, 512], f32, tag="ow", name=f"ow{sc}_{dc}")
                nc.vector.tensor_copy(out=ow, in_=pp)
                store_q[nst % 3].dma_start(out=P_t[ssl, dsl], in_=ow)
                nst += 1

        nc.gpsimd.collective_compute(
            "ReduceScatter",
            mybir.AluOpType.add,
            replica_groups=[[0, 1, 2, 3], [4, 5, 6, 7]],
            ins=[P_t.opt()],
            outs=[R_t.opt()],
        )

        # quantize R (f32) -> int8 with per-row scale 126/absmax; the DVE
        # f32->int8 copy rounds to nearest (verified on hw: rel err matches
        # the round-to-nearest emulation, 8.3e-3, not truncation's 1.6e-2)
        for rc in range(RS // 128):
            rsl = slice(rc * 128, (rc + 1) * 128)
            rf = pout.tile([128, DIM], f32, tag="rf", bufs=2, name=f"rf{rc}")
            nc.sync.dma_start(out=rf, in_=R_t[rsl])
            mx = pout.tile([128, 1], f32, tag="mx", bufs=2, name=f"mx{rc}")
            nc.vector.tensor_reduce(out=mx, in_=rf, axis=mybir.AxisListType.X,
                                    op=mybir.AluOpType.max,
                                    apply_absolute_value=True)
            nc.scalar.dma_start(out=out_d[rsl, DIM:DIM + 4],
                                in_=mx.bitcast(mybir.dt.int8))
            rcp = pout.tile([128, 1], f32, tag="rcp", bufs=2, name=f"rcp{rc}")
            nc.vector.reciprocal(out=rcp, in_=mx)
            sc = pout.tile([128, 1], f32, tag="sc", bufs=2, name=f"sc{rc}")
            nc.vector.tensor_scalar_mul(out=sc, in0=rcp, scalar1=126.0)
            t = pout.tile([128, DIM], f32, tag="t", bufs=2, name=f"t{rc}")
            nc.vector.tensor_scalar_mul(out=t, in0=rf, scalar1=sc)
            qb = pout.tile([128, DIM], mybir.dt.int8, tag="qb", bufs=2,
                           name=f"qb{rc}")
            nc.vector.tensor_copy(out=qb, in_=t)
            nc.gpsimd.dma_start(out=out_d[rsl, 0:DIM], in_=qb)

    _split_excess_waits(nc)
    return nc


def _split_excess_waits(nc):
    import concourse.mybir as mybir

    n = 0
    for func in nc.m.functions:
        for block in func.blocks:
            out = []
            for inst in block.instructions:
                si = getattr(inst, "sync_info", None)
                if si is not None and si.on_wait and len(si.on_wait) > 1:
                    for w in si.on_wait[:-1]:
                        nop = mybir.InstNoOp(
                            name=f"wsplit_{n}",
                            engine=inst.engine,
                            sync_info=mybir.SyncInfo(on_wait=[w], on_update=[]),
                            bass_nofuse=True,
                        )
                        n += 1
                        out.append(nop)
                    inst.sync_info = mybir.SyncInfo(
                        on_wait=[si.on_wait[-1]], on_update=si.on_update)
                out.append(inst)
            block.instructions[:] = out
    return n


_PREP_BUFS = None
_CAST_JIT = None


def _prep_all(q, k, v, Wq, Wk, Wv, bq, bk, bv, Wo, bf16, fp8):
    """Host prep: per-head BLAS GEMMs into persistent f32 buffers, then one
    jax-cpu jit for bias-add + Vc chunk layout + fp8/bf16 casts (XLA's
    vectorized convert is ~3x faster than ml_dtypes.astype for fp8)."""
    global _PREP_BUFS, _CAST_JIT
    QTf, KTf, Vf = _PREP_BUFS if _PREP_BUFS is not None else (
        np.empty((B, H, 128, S), np.float32),
        np.empty((B, H, 128, S), np.float32),
        np.empty((B, H, S, D), np.float32))
    _PREP_BUFS = (QTf, KTf, Vf)
    for b in range(B):
        for h in range(H):
            np.matmul(Wq[h].T, q[b, h].T, out=QTf[b, h])
            np.matmul(Wk[h].T, k[b, h].T, out=KTf[b, h])
            np.matmul(v[b, h], Wv[h], out=Vf[b, h])
    if _CAST_JIT is None:
        try:
            import jax
            co = jax.lax.convert_element_type

            def _f(QT, KT, V, bq_, bk_, bv_, Wo_):
                import jax.numpy as jnp

                QT = QT + bq_[None, :, :, None]
                KT = KT + bk_[None, :, :, None]
                V = V + bv_[None, :, None, :]
                Vc = V.reshape(B, H, NKT, 128, D).transpose(
                    0, 1, 3, 2, 4).reshape(B, H, 128, S)
                QKV = jnp.stack([QT, KT, Vc], axis=2)  # [B,H,3,128,S]
                return co(QKV, fp8), co(Wo_.reshape(H, 128, DIM), bf16)

            _CAST_JIT = jax.jit(_f, device=jax.devices("cpu")[0])
            _CAST_JIT(QTf, KTf, Vf, bq, bk, bv, Wo)
        except Exception:
            _CAST_JIT = "numpy"
    if _CAST_JIT == "numpy":
        QT8 = (QTf + bq[None, :, :, None]).astype(fp8)
        KT8 = (KTf + bk[None, :, :, None]).astype(fp8)
        Vc8 = (Vf + bv[None, :, None, :]).reshape(B, H, NKT, 128, D).transpose(
            0, 1, 3, 2, 4).reshape(B, H, 128, S).astype(fp8)
        QKV8 = np.stack([QT8, KT8, Vc8], axis=2)
        Wo_b = Wo.reshape(H, 128, DIM).astype(bf16)
    else:
        QKV8, Wo_b = (np.asarray(x) for x in
                      _CAST_JIT(QTf, KTf, Vf, bq, bk, bv, Wo))
    return QKV8, Wo_b


def _prep_core(c, QKV8, Wo_b):
    b = c // 4
    h0 = (c % 4) * HPC
    jh = h0 if b == 0 else h0 + HPC // 2
    return {"qkv": QKV8[b, h0:h0 + HPC],
            "woh": Wo_b[jh:jh + HPC // 2]}


_CACHE_SET = False


def _enable_jax_compile_cache():
    global _CACHE_SET
    if _CACHE_SET:
        return
    try:
        import jax

        jax.config.update("jax_compilation_cache_dir", "/tmp/jax_comp_cache")
        jax.config.update("jax_persistent_cache_min_compile_time_secs", 0)
        jax.config.update("jax_persistent_cache_min_entry_size_bytes", 0)
    except Exception:
        pass
    _CACHE_SET = True


def kernel(q, k, v, Wq, Wk, Wv, bq, bk, bv, Wo, bo):
    global _BUILT, LAST_RESULTS
    _import_concourse()
    _enable_jax_compile_cache()
    import ml_dtypes

    from concourse.bass_utils import run_bass_kernel_spmd

    bf16 = ml_dtypes.bfloat16
    fp8 = ml_dtypes.float8_e3m4
    args = [np.asarray(x, dtype=np.float32)
            for x in (q, k, v, Wq, Wk, Wv, bq, bk, bv)]
    Wo = np.asarray(Wo, dtype=np.float32)
    bo = np.asarray(bo, dtype=np.float32)
    if _BUILT is None:
        _BUILT = _build()
    prepped = _prep_all(*args, Wo, bf16, fp8)
    in_maps = [_prep_core(c, *prepped) for c in range(NC)]
    res = run_bass_kernel_spmd(_BUILT, in_maps, core_ids=list(range(NC)),
                               trace=TRACE)
    LAST_RESULTS = res
    out = np.empty((B, S, DIM), dtype=np.float32)
    for c in range(NC):
        b = c // 4
        g = c % 4
        buf = np.asarray(res.results[c]["outq"])  # [RS, DIM+4] int8
        deq = buf[:, DIM:DIM + 4].copy().view(np.float32) / 126.0  # [RS,1]
        np.multiply(buf[:, 0:DIM].astype(np.float32), deq,
                    out=out[b, g * RS:(g + 1) * RS])
    out += bo
    return out
